# revision 8
# baseline (speedup 1.0000x reference)
"""Trainium2 Bass kernel for GQA attention (B=2, L=2048, D=2048, H=16, KV=8, HD=128).

Sharding: 2-way data-parallel over batch x 4-way tensor-parallel over heads
(KV-head groups intact). Each core handles one batch and 4 query heads
(2 KV heads): QKV projection + RoPE + RMSNorm + flash-style attention +
a PARTIAL output projection over its 4 heads' rows of wo (row-sharded wo).
The host gather SUMS the 4 partial [D, L] outputs per batch — no on-device
collective. Avoiding collective_compute keeps the PE at full clock
(~216ns per 128x128x512 fp16 matmul vs ~263ns with a collective armed).

All heavy matmuls are fp16 with fp32 PSUM accumulation. rstd and softmax
denominators use the Ln/Exp one-table trick and PE ones/broadcast matmuls.
"""
import math
import numpy as np

B, L, D = 2, 2048, 2048
H, KV, HD = 16, 8, 128
NCORES = 8
HPC = 4              # query heads per core
KPC = 2              # kv heads per core
EPS = 1e-5
ROPE_BASE = 10000.0
SCALE = HD ** -0.5

TT = 512             # token tile (free dim)
NTT = L // TT        # 4 token tiles
NDC = D // 128       # 16 contraction chunks in qkv proj
NFC = 8              # 4 q + 2 k + 2 v column chunks of 128
QKV_COLS = NFC * 128

_CACHE = {}


def _rope_tables():
    """cos/sin LUTs [64, L] computed exactly like the jax reference (f32, cpu)."""
    import jax
    import jax.numpy as jnp

    cpu = jax.devices("cpu")[0]
    with jax.default_device(cpu):
        base = ROPE_BASE * 1.0 ** (HD / (HD - 2))
        freqs = base ** (jnp.arange(0, HD, 2, dtype=jnp.float32) / HD)   # [64]
        pos = jnp.arange(L, dtype=jnp.float32)                           # [L]
        angles = pos[:, None] * freqs[None, :]                           # [L, 64]
        cos = np.asarray(jnp.cos(angles), dtype=np.float32).T.copy()     # [64, L]
        sin = np.asarray(jnp.sin(angles), dtype=np.float32).T.copy()
    return cos, sin


def _build_nc():
    import concourse.bass as bass
    import concourse.tile as tile
    import concourse.mybir as mybir
    from concourse import bacc
    from concourse.masks import make_identity
    from contextlib import ExitStack

    f32 = mybir.dt.float32
    f16 = mybir.dt.float16
    Exp = mybir.ActivationFunctionType.Exp
    Ln = mybir.ActivationFunctionType.Ln
    mult = mybir.AluOpType.mult
    add = mybir.AluOpType.add
    sub = mybir.AluOpType.subtract

    from concourse import bacc as _bacc_mod

    if not getattr(_bacc_mod, "_act_table_patch", False):
        _orig_get = _bacc_mod.get_activation_tables

        def _patched_get(arch):
            t = _orig_get(arch)
            exp = mybir.ActivationFunctionType.Exp
            ln = mybir.ActivationFunctionType.Ln
            for name, funcs in t.items():
                if name != "natural_log_exp_and_others":
                    funcs.discard(exp)
                    funcs.discard(ln)
            return t

        _bacc_mod.get_activation_tables = _patched_get
        _bacc_mod._act_table_patch = True

    nc = bacc.Bacc(num_devices=NCORES)

    # per-core inputs (host pre-sliced)
    xT = nc.dram_tensor("xT", [D, L], f16, kind="ExternalInput")
    wqkv = nc.dram_tensor("wqkv", [D, QKV_COLS], f16, kind="ExternalInput")
    # wo rows for this core's 4 heads, tiled [128, h, oc, 128]
    woT = nc.dram_tensor("woT", [128, HPC * NDC * 128], f16, kind="ExternalInput")
    lcos = nc.dram_tensor("lcos", [64, L], f16, kind="ExternalInput")
    lsin = nc.dram_tensor("lsin", [64, L], f16, kind="ExternalInput")
    qn = nc.dram_tensor("qn", [HD, 1], f32, kind="ExternalInput")
    kn = nc.dram_tensor("kn", [HD, 1], f32, kind="ExternalInput")
    # partial output: yT[oc, tok] = sum over this core's heads
    yT = nc.dram_tensor("yT", [D, L], f32, kind="ExternalOutput")

    with tile.TileContext(nc) as tc, ExitStack() as ctx, nc.allow_low_precision(
        reason="f16 storage; all matmul accumulation is fp32 PSUM"
    ):
        consts = ctx.enter_context(tc.tile_pool(name="consts", bufs=1))
        qkvp = ctx.enter_context(tc.tile_pool(name="qkvp", bufs=1))
        ropep = ctx.enter_context(tc.tile_pool(name="ropep", bufs=2))
        halfp = ctx.enter_context(tc.tile_pool(name="halfp", bufs=4))
        statp = ctx.enter_context(tc.tile_pool(name="statp", bufs=3))
        sap = ctx.enter_context(tc.tile_pool(name="sap", bufs=5))
        expp = ctx.enter_context(tc.tile_pool(name="expp", bufs=5))
        attp = ctx.enter_context(tc.tile_pool(name="attp", bufs=2))
        bcp = ctx.enter_context(tc.tile_pool(name="bcp", bufs=3))
        yp = ctx.enter_context(tc.tile_pool(name="yp", bufs=2))

        pacc = ctx.enter_context(tc.tile_pool(name="pacc", bufs=2, space="PSUM"))
        pstream = ctx.enter_context(tc.tile_pool(name="pstream", bufs=3, space="PSUM"))
        pout = ctx.enter_context(tc.tile_pool(name="pout", bufs=2, space="PSUM"))
        pmisc = ctx.enter_context(tc.tile_pool(name="pmisc", bufs=1, space="PSUM"))

        # ---- constants ----
        ones_f = consts.tile([128, 1], f32)
        nc.vector.memset(ones_f, 1.0)
        ones = consts.tile([128, 1], f16)
        nc.vector.tensor_copy(out=ones, in_=ones_f)
        ones_k1_f = consts.tile([1, 128], f32)
        nc.vector.memset(ones_k1_f, 1.0)
        ones_k1 = consts.tile([1, 128], f16)
        nc.vector.tensor_copy(out=ones_k1, in_=ones_k1_f)
        ident = consts.tile([128, 128], f16)
        make_identity(nc, ident)
        eps_t = consts.tile([1, 1], f32)
        nc.vector.memset(eps_t, EPS)
        # ---- resident tensors (per-dc interleaved so compute starts early) ----
        x_sb = consts.tile([128, NDC, L], f16)
        _xr = xT.ap().rearrange("(dc p) t -> p dc t", p=128)
        w_sb = consts.tile([128, NDC, QKV_COLS], f16)
        _wr = wqkv.ap().rearrange("(dc p) f -> p dc f", p=128)
        for dc in range(NDC):
            nc.sync.dma_start(out=w_sb[:, dc:dc + 1, :], in_=_wr[:, dc:dc + 1, :])
            nc.sync.dma_start(out=x_sb[:, dc:dc + 1, :], in_=_xr[:, dc:dc + 1, :])
        cos_sb = consts.tile([128, L], f16)
        nc.gpsimd.dma_start(out=cos_sb[0:64, :], in_=lcos[:, :])
        nc.gpsimd.dma_start(out=cos_sb[64:128, :], in_=lcos[:, :])
        sin_sb = consts.tile([128, L], f16)
        nc.gpsimd.dma_start(out=sin_sb[0:64, :], in_=lsin[:, :])
        nc.gpsimd.dma_start(out=sin_sb[64:128, :], in_=lsin[:, :])
        qn_sb = consts.tile([HD, 1], f32)
        nc.gpsimd.dma_start(out=qn_sb, in_=qn[:, :])
        kn_sb = consts.tile([HD, 1], f32)
        nc.gpsimd.dma_start(out=kn_sb, in_=kn[:, :])
        # wo tiles [128, h, oc, 128] (needed only at out-projection)
        wo_sb = consts.tile([128, HPC, NDC, 128], f16)
        _wor = woT.ap().rearrange("p (h oc m) -> p h oc m", h=HPC, oc=NDC)
        nc.sync.dma_start(out=wo_sb[:, 0:2, :, :], in_=_wor[:, 0:2, :, :])
        nc.sync.dma_start(out=wo_sb[:, 2:4, :, :], in_=_wor[:, 2:4, :, :])

        # activations
        qh_t = [qkvp.tile([128, L], f16, name=f"qh{h}") for h in range(HPC)]
        kh_t = [qkvp.tile([128, L], f16, name=f"kh{g}") for g in range(KPC)]
        v_t = [qkvp.tile([128, L // 128, HD], f16, name=f"v{g}") for g in range(KPC)]
        a_t = [qkvp.tile([128, L], f16, name=f"a{h}") for h in range(HPC)]

        def proj_tt(tt, fcs):
            """QKV projection + RoPE + RMSNorm for one 512-token tile, given fc list."""
            pos0 = tt * TT
            for fc in fcs:
                pp = pacc.tile([128, TT], f32, tag="pacc", name=f"pp{fc}")
                for dc in range(NDC):
                    nc.tensor.matmul(
                        pp,
                        w_sb[:, dc, fc * 128:(fc + 1) * 128],
                        x_sb[:, dc, pos0:pos0 + TT],
                        start=(dc == 0),
                        stop=(dc == NDC - 1),
                    )
                if fc < 6:
                    # q heads (fc 0-3) and k heads (fc 4-5): rope + rmsnorm
                    rsrc = ropep.tile([128, TT], f16, tag="rsrc")
                    nc.vector.tensor_copy(out=rsrc, in_=pp)
                    cs_lo = cos_sb[0:64, pos0:pos0 + TT]
                    cs_hi = cos_sb[64:128, pos0:pos0 + TT]
                    sn_lo = sin_sb[0:64, pos0:pos0 + TT]
                    sn_hi = sin_sb[64:128, pos0:pos0 + TT]
                    x1 = rsrc[0:64, :]
                    x2 = rsrc[64:128, :]
                    t1 = halfp.tile([64, TT], f16, tag="half")
                    t2 = halfp.tile([64, TT], f16, tag="half")
                    t3 = halfp.tile([64, TT], f16, tag="half")
                    t4 = halfp.tile([64, TT], f16, tag="half")
                    roped = ropep.tile([128, TT], f16, tag="roped")
                    nc.vector.tensor_tensor(out=t1, in0=x1, in1=cs_lo, op=mult)
                    nc.vector.tensor_tensor(out=t2, in0=x2, in1=sn_hi, op=mult)
                    nc.vector.tensor_tensor(out=roped[0:64, :], in0=t1, in1=t2, op=sub)
                    nc.vector.tensor_tensor(out=t3, in0=x2, in1=cs_hi, op=mult)
                    nc.vector.tensor_tensor(out=t4, in0=x1, in1=sn_lo, op=mult)
                    nc.vector.tensor_tensor(out=roped[64:128, :], in0=t3, in1=t4, op=add)
                    sq = ropep.tile([128, TT], f16, tag="sq")
                    nc.vector.tensor_tensor(out=sq, in0=roped, in1=roped, op=mult)
                    pss = pstream.tile([1, TT], f32, tag="pstream")
                    nc.tensor.matmul(pss, ones, sq, start=True, stop=True)
                    lnt = statp.tile([1, TT], f32, tag="stat")
                    nc.scalar.activation(
                        out=lnt, in_=pss, func=Ln, bias=eps_t, scale=1.0 / HD
                    )
                    srd = statp.tile([1, TT], f16, tag="stat")
                    nc.scalar.activation(out=srd, in_=lnt, func=Exp, scale=-0.5)
                    pb = bcp.tile([128, TT], f16, tag="bc")
                    nc.gpsimd.partition_broadcast(out_ap=pb, in_ap=srd)
                    w_head = qn_sb if fc < 4 else kn_sb
                    if fc < 4:
                        dst = qh_t[fc][:, pos0:pos0 + TT]
                    else:
                        dst = kh_t[fc - 4][:, pos0:pos0 + TT]
                    nc.vector.scalar_tensor_tensor(
                        out=dst, in0=roped, scalar=w_head, in1=pb,
                        op0=mult, op1=mult,
                    )
                else:
                    # v heads (fc 6-7): transpose to [tok, HD]
                    g = fc - 6
                    vt = ropep.tile([128, TT], f16, tag="rsrc")
                    nc.vector.tensor_copy(out=vt, in_=pp)
                    for i in range(TT // 128):
                        pt = pstream.tile([128, 128], f16, tag="pstream")
                        nc.tensor.transpose(pt, vt[:, i * 128:(i + 1) * 128], ident)
                        nc.vector.tensor_copy(out=v_t[g][:, tt * 4 + i, :], in_=pt)

        def att_tiles(jobs):
            """Interleaved attention for (h, tqt) query tiles (same kv group)."""
            NK = L // 128
            state = []
            for h, tqt in jobs:
                g = h // 2
                qs = qh_t[h][:, tqt * TT:(tqt + 1) * TT]
                po = pacc.tile([128, TT], f32, tag="pacc", name=f"po{h}{tqt}")
                sacc = [
                    sap.tile([128, TT], f16, tag="sacc", name=f"sa{p}{h}{tqt}")
                    for p in range(2)
                ]
                state.append((h, g, tqt, qs, po, sacc))
            for tk in range(NK):
                ets = []
                for h, g, tqt, qs, po, sacc in state:
                    ps = pstream.tile([128, TT], f32, tag="pstream")
                    nc.tensor.matmul(
                        ps, kh_t[g][:, tk * 128:(tk + 1) * 128], qs,
                        start=True, stop=True,
                    )
                    et = expp.tile([128, TT], f16, tag="expt")
                    nc.scalar.activation(out=et, in_=ps, func=Exp, scale=SCALE)
                    ets.append(et)
                for (h, g, tqt, qs, po, sacc), et in zip(state, ets):
                    nc.tensor.matmul(
                        po, v_t[g][:, tk, :], et,
                        start=(tk == 0), stop=(tk == NK - 1),
                    )
                    sa = sacc[tk % 2]
                    if tk < 2:
                        nc.vector.tensor_copy(out=sa, in_=et)
                    else:
                        nc.vector.tensor_tensor(out=sa, in0=sa, in1=et, op=add)
            for h, g, tqt, qs, po, sacc in state:
                pd = pmisc.tile([1, TT], f32, tag="pmisc")
                nc.tensor.matmul(pd, ones, sacc[0], start=True, stop=False)
                nc.tensor.matmul(pd, ones, sacc[1], start=False, stop=True)
                rdf = statp.tile([1, TT], f32, tag="stat")
                nc.vector.reciprocal_approx_fast(out=rdf, in_=pd)
                rd = statp.tile([1, TT], f16, tag="stat")
                nc.vector.tensor_copy(out=rd, in_=rdf)
                pb = bcp.tile([128, TT], f16, tag="bc")
                nc.gpsimd.partition_broadcast(out_ap=pb, in_ap=rd)
                o_sb = attp.tile([128, TT], f32, tag="att")
                nc.vector.tensor_copy(out=o_sb, in_=po)
                nc.vector.tensor_tensor(
                    out=a_t[h][:, tqt * TT:(tqt + 1) * TT], in0=o_sb, in1=pb, op=mult
                )

        def outproj_tt(tt):
            """Partial out-projection for one token tile over this core's 4 heads."""
            pos0 = tt * TT
            for oc in range(NDC):
                py = pout.tile([128, TT], f32, tag="pout", name=f"py{oc % 2}")
                for h in range(HPC):
                    nc.tensor.matmul(
                        py, wo_sb[:, h, oc, :], a_t[h][:, pos0:pos0 + TT],
                        start=(h == 0), stop=(h == HPC - 1),
                    )
                yt = yp.tile([128, TT], f32, tag="y")
                nc.vector.tensor_copy(out=yt, in_=py)
                nc.sync.dma_start(
                    out=yT[oc * 128:(oc + 1) * 128, pos0:pos0 + TT], in_=yt
                )

        # ---- schedule ----
        # K/V projection first; then Q proj interleaved with attention so the
        # ACT exp stream always trails the PE; out-projection trails attention.
        for tt in range(NTT):
            proj_tt(tt, [4, 5, 6, 7])
        proj_tt(0, [0, 1, 2, 3])
        att_tiles([(0, 0), (1, 0)])
        proj_tt(1, [0, 1, 2, 3])
        att_tiles([(2, 0), (3, 0)])
        proj_tt(2, [0, 1, 2, 3])
        att_tiles([(0, 1), (1, 1)])
        proj_tt(3, [0, 1, 2, 3])
        att_tiles([(2, 1), (3, 1)])
        att_tiles([(0, 2), (1, 2)])
        outproj_tt(0)
        att_tiles([(2, 2), (3, 2)])
        outproj_tt(1)
        att_tiles([(0, 3), (1, 3)])
        att_tiles([(2, 3), (3, 3)])
        outproj_tt(2)
        outproj_tt(3)

    nc.finalize()
    return nc


def kernel(x, wq, wk, wv, wo, qn_w, kn_w):
    from concourse.bass_utils import run_bass_kernel_spmd

    if "nc" not in _CACHE:
        _CACHE["nc"] = _build_nc()
    nc = _CACHE["nc"]

    x = np.asarray(x, dtype=np.float32)
    wq = np.asarray(wq, dtype=np.float32)
    wk = np.asarray(wk, dtype=np.float32)
    wv = np.asarray(wv, dtype=np.float32)
    wo = np.asarray(wo, dtype=np.float32)
    qn_w = np.asarray(qn_w, dtype=np.float32).reshape(HD, 1).copy()
    kn_w = np.asarray(kn_w, dtype=np.float32).reshape(HD, 1).copy()

    cos, sin = _rope_tables()
    cos = cos.astype(np.float16)
    sin = sin.astype(np.float16)

    in_maps = []
    for c in range(NCORES):
        bc, hc4 = divmod(c, 4)
        xT_c = np.ascontiguousarray(x[bc].T.astype(np.float16))
        wqkv_c = np.ascontiguousarray(
            np.concatenate(
                [
                    wq[:, hc4 * HPC * HD:(hc4 + 1) * HPC * HD],
                    wk[:, hc4 * KPC * HD:(hc4 + 1) * KPC * HD],
                    wv[:, hc4 * KPC * HD:(hc4 + 1) * KPC * HD],
                ],
                axis=1,
            ).astype(np.float16)
        )
        # wo rows for this core's heads -> [128, h*oc*128]
        wo_sl = wo[hc4 * HPC * HD:(hc4 + 1) * HPC * HD, :].astype(np.float16)
        wo_t = np.ascontiguousarray(
            wo_sl.reshape(HPC, 128, NDC, 128).transpose(1, 0, 2, 3).reshape(
                128, HPC * NDC * 128
            )
        )
        in_maps.append(
            {
                "xT": xT_c,
                "wqkv": wqkv_c,
                "woT": wo_t,
                "lcos": cos,
                "lsin": sin,
                "qn": qn_w,
                "kn": kn_w,
            }
        )

    trace = bool(_CACHE.get("trace"))
    r = run_bass_kernel_spmd(
        nc, in_maps, core_ids=list(range(NCORES)), trace=trace
    )
    _CACHE["last_result"] = r

    y = np.empty((B, L, D), dtype=np.float32)
    for bc in range(B):
        acc = r.results[bc * 4]["yT"].copy()
        for hc4 in range(1, 4):
            acc += r.results[bc * 4 + hc4]["yT"]
        y[bc] = acc.T
    return y
